# revision 17
# baseline (speedup 1.0000x reference)
"""Conv-QKV self-attention (CSA) Trainium2 Bass kernel.

Reference computation (per batch b):
    k = conv1d(x, K_w, K_b); q = conv1d(x, Q_w, Q_b); v = conv1d(x, V_w, V_b)
    scores = relu(k^T q)                # [L, L], contraction over 64 channels
    out = v @ scores / sqrt(3)          # [64, L], contraction over L
Sharding: 8 cores = 4 batches x 2 row-halves (l) of the score matrix.
Each core computes k, vT for its l-half, q for the full L, a flash-style
pass over relu(k^T q) tiles, and a PARTIAL out (contraction over its
l-half).  The host sums the two partials per batch.  1/sqrt(3) is folded
into the V weights on the host.

v3 design notes (HW-trace driven):
 - everything bf16 on the PE (1 cyc/row; f32r is 2 on HW).  rel err
   ~3e-3 vs the 2e-2 gate.
 - the HAM clock gate only reaches 8/8 (2.4 GHz + MM pipelining) after
   ~3.4us of *dense* PE activity; a stalling conv phase keeps the whole
   kernel at 4/8.  So: a short warmup burst covers the DMA preamble and
   the conv phase never blocks on the Act engine (psum bufs=4, copies
   split Act/DVE).
 - K and Q convs over the core's own l-half are fused into ONE stationary
   [128, 128] (K-cols 0:64, Q-cols 64:128): one moving-x stream computes
   both.  Taps 0+1 are K-stacked on partitions (host ships x with a
   1-shifted copy on partitions 64:128), tap 2 is a K=64 second matmul.
 - x is shipped as two 2050-col blocks [own-half | other-half] so the
   same SPMD program works for both half-cores; q2/out columns are in
   block-permuted order and the host un-permutes.
 - flash loop: row-packed mm1 pairs (two K=64 l-tiles on row groups
   h0/h64 run concurrently), depth-2 software pipeline for the mm2
   accumulation, relu alternating DVE/Act, scores pool bufs=3.
"""

import numpy as np

FIN, FOUT, KS = 64, 64, 3
B, L = 4, 4096
HALF = L // 2            # per-core l range
NCORES = 8
MT = 512                 # m tile (PSUM bank free dim, fp32)
LT = 128                 # l tile (PE partition dim)
N_MT = L // MT           # 8  (full m range per core, block-permuted)
N_LT = HALF // LT        # 16 (l tiles in this core's half)
N_G = HALF // MT         # 4  (conv groups per 2048-col block)
BLK = HALF + 2           # 2050: x block incl +-1 halo
SQRT_KS = float(np.sqrt(KS))
N_WARM = 12              # PE warmup matmuls (HAM ramp + DMA preamble cover)

_NC_CACHE = {}


def _build_nc():
    from contextlib import ExitStack

    import concourse.tile as tile
    from concourse import bacc, mybir

    f32 = mybir.dt.float32
    bf16 = mybir.dt.bfloat16
    AF = mybir.ActivationFunctionType

    nc = bacc.Bacc("TRN2", target_bir_lowering=False)

    # x as two 2050-col halo blocks [own | other], each with a 1-shifted
    # copy on partitions 64:128 (K-stacked taps 0+1).  DMA-ing raw bf16.
    xd_d = nc.dram_tensor("xd", [128, 2 * BLK], bf16, kind="ExternalInput")
    # fused conv weights [128, 256]:
    #   [:,   0: 64] = [Kt0;Kt1]   [:,  64:128] = [Qt0;Qt1]
    #   [0:64,128:192] = Kt2       [0:64,192:256] = Qt2   (rows 64:128 zero)
    kqw_d = nc.dram_tensor("kqw", [128, 4 * FOUT], bf16, kind="ExternalInput")
    # v weights (1/sqrt(3) folded): [:,0:64] = [Vt0;Vt1], [0:64,64:128] = Vt2
    vw_d = nc.dram_tensor("vw", [128, 2 * FOUT], bf16, kind="ExternalInput")
    # col 0 = [K_b; Q_b]; col 1 = [Q_b; 0] (partition-0:64-aligned copy)
    kqb_d = nc.dram_tensor("kqb", [128, 2], f32, kind="ExternalInput")
    vb_d = nc.dram_tensor("vb", [1, FOUT], f32, kind="ExternalInput")
    out_d = nc.dram_tensor("out", [FOUT, L], f32, kind="ExternalOutput")

    with tile.TileContext(nc) as tc, ExitStack() as ctx:
        consts = ctx.enter_context(tc.tile_pool(name="consts", bufs=1))
        big = ctx.enter_context(tc.tile_pool(name="big", bufs=1))

        # ---- DMA preamble (weights first: tiny, gate the convs) -------
        kqw_sb = consts.tile([128, 4 * FOUT], bf16)
        nc.sync.dma_start(out=kqw_sb, in_=kqw_d[:, :])
        vw_sb = consts.tile([128, 2 * FOUT], bf16)
        nc.sync.dma_start(out=vw_sb, in_=vw_d[:, :])
        kqb_sb = consts.tile([128, 2], f32)
        nc.sync.dma_start(out=kqb_sb, in_=kqb_d[:, :])
        vb_sb = consts.tile([128, FOUT], f32)
        nc.sync.dma_start(out=vb_sb, in_=vb_d[:, :].to_broadcast([128, FOUT]))

        # x blocks, chunked so conv group g can start once its chunk lands
        xd_sb = consts.tile([128, 2 * BLK], bf16)
        for c in range(4):                       # own block: 4 chunks on sync q
            sl = slice(c * MT, (c + 1) * MT + (2 if c == 3 else 0))
            nc.sync.dma_start(out=xd_sb[:, sl], in_=xd_d[:, sl])
        for c in range(4):                       # other block: gpsimd q
            lo = BLK + c * MT
            sl = slice(lo, lo + MT + (2 if c == 3 else 0))
            nc.gpsimd.dma_start(out=xd_sb[:, sl], in_=xd_d[:, sl])

        # conv outputs, duplicated on both partition halves for the
        # row-packed score matmuls
        k2_sb = big.tile([128, HALF], bf16)
        q2_sb = big.tile([128, L], bf16)
        vt_sb = big.tile([128, N_LT, FOUT], bf16)

        # ---- stage A: warmup + conv projections ----------------------
        actx = ctx.enter_context(ExitStack())
        cpool = actx.enter_context(tc.tile_pool(name="cpsum", bufs=4, space="PSUM"))
        vpool = actx.enter_context(tc.tile_pool(name="vpsum", bufs=2, space="PSUM"))

        # All-engine warmup burst on the first landed x chunk: the clock
        # gate watches chip-wide switching activity (zero/constant data
        # does NOT count), and with every engine hot it reaches 8/8 in
        # ~5.6us instead of ~30us.  Uses real (random-normal) x data.
        wscr = consts.tile([128, MT], bf16)
        for i in range(N_WARM):
            wp = cpool.tile([128, MT], f32, name="wp", tag="cp")
            # stays inside DMA chunk 0 (cols 0:512) so it starts ASAP
            nc.tensor.matmul(
                wp[:, 0:384], xd_sb[:, 0:128], xd_sb[:, 128:512],
                start=True, stop=True,
            )
            if i % 3 == 0:
                nc.vector.tensor_scalar_max(wscr, xd_sb[:, 0:MT], 0.0)
            elif i % 3 == 1:
                nc.scalar.activation(wscr, xd_sb[:, 0:MT], AF.Relu)
            else:
                nc.gpsimd.tensor_scalar_max(wscr[:, 0:128], xd_sb[:, 0:128], 0.0)

        # fused K+Q conv over the own block -> k2 (all) + q2 cols 0:2048
        for g in range(N_G):
            p = cpool.tile([128, MT], f32, name="pkq", tag="cp")
            nc.tensor.matmul(
                p, kqw_sb[:, 0:128], xd_sb[:, g * MT : g * MT + MT],
                start=True, stop=False,
            )
            nc.tensor.matmul(
                p, kqw_sb[0:FIN, 128:256], xd_sb[0:FIN, g * MT + 2 : g * MT + 2 + MT],
                start=False, stop=True,
            )
            gsl = slice(g * MT, (g + 1) * MT)
            nc.scalar.activation(
                k2_sb[0:FOUT, gsl], p[0:FOUT, :], AF.Identity, bias=kqb_sb[0:FOUT, 0:1]
            )
            nc.vector.tensor_scalar_add(
                q2_sb[FOUT:128, gsl], p[FOUT:128, :], kqb_sb[FOUT:128, 0:1]
            )
            nc.sync.dma_start(out=k2_sb[FOUT:128, gsl], in_=k2_sb[0:FOUT, gsl])
            nc.gpsimd.dma_start(out=q2_sb[0:FOUT, gsl], in_=q2_sb[FOUT:128, gsl])

        # vT tiles: [HALF, 64] in 128-row tiles, taps 0+1 K-stacked
        for j in range(N_LT):
            pv = vpool.tile([128, FOUT], f32, name="pv", tag="pv")
            nc.tensor.matmul(
                pv, xd_sb[:, j * LT : j * LT + LT], vw_sb[:, 0:FOUT],
                start=True, stop=False,
            )
            nc.tensor.matmul(
                pv, xd_sb[0:FIN, j * LT + 2 : j * LT + 2 + LT], vw_sb[0:FIN, FOUT:128],
                start=False, stop=True,
            )
            nc.vector.tensor_add(vt_sb[:, j, :], pv, vb_sb)

        # Q-only conv over the other block -> q2 cols 2048:4096
        for g in range(N_G):
            pq = cpool.tile([128, MT], f32, name="pq", tag="cp")
            p = pq[0:FOUT, :]
            lo = BLK + g * MT
            nc.tensor.matmul(
                p, kqw_sb[:, 64:128], xd_sb[:, lo : lo + MT], start=True, stop=False
            )
            nc.tensor.matmul(
                p, kqw_sb[0:FIN, 192:256], xd_sb[0:FIN, lo + 2 : lo + 2 + MT],
                start=False, stop=True,
            )
            gsl = slice(HALF + g * MT, HALF + (g + 1) * MT)
            if g % 2 == 0:
                nc.scalar.activation(
                    q2_sb[0:FOUT, gsl], p, AF.Identity, bias=kqb_sb[0:FOUT, 1:2]
                )
            else:
                nc.vector.tensor_scalar_add(q2_sb[0:FOUT, gsl], p, kqb_sb[0:FOUT, 1:2])
            nc.sync.dma_start(out=q2_sb[FOUT:128, gsl], in_=q2_sb[0:FOUT, gsl])

        # ---- stage B: flash loop over score tiles --------------------
        actx.close()
        spsum = ctx.enter_context(tc.tile_pool(name="spsum", bufs=3, space="PSUM"))
        spool = ctx.enter_context(tc.tile_pool(name="spool", bufs=4))
        opsum = ctx.enter_context(tc.tile_pool(name="opsum", bufs=2, space="PSUM"))
        opool = ctx.enter_context(tc.tile_pool(name="opool", bufs=2))

        NPAIR = N_LT // 2  # adjacent-l-tile pairs per m-tile
        for mt in range(N_MT):
            po = opsum.tile([FOUT, MT], f32, name="po")
            pending = []  # [(pair, s_sb), ...] awaiting their mm2 pairs

            def flush_mm2(last=False):
                p, ps_sb = pending.pop(0)
                nc.tensor.matmul(
                    po, vt_sb[:, 2 * p, :], ps_sb[:, 0:MT],
                    start=(p == 0), stop=False,
                )
                nc.tensor.matmul(
                    po, vt_sb[:, 2 * p + 1, :], ps_sb[:, MT : 2 * MT],
                    start=False, stop=last,
                )

            # batch two mm1 pairs, then two mm2 singles-blocks: fewer
            # pair<->single PE mode transitions (each costs a fill/drain)
            for pp in range(0, NPAIR, 2):
                for p in (pp, pp + 1):
                    ltA, ltB = 2 * p, 2 * p + 1
                    ps = spsum.tile([128, 2 * MT], f32, name="ps")
                    # mm1 pair: row-packed (K=64 each) over adjacent l-tiles
                    nc.tensor.matmul(
                        ps[:, 0:MT],
                        k2_sb[0:FOUT, ltA * LT : (ltA + 1) * LT],
                        q2_sb[0:FOUT, mt * MT : (mt + 1) * MT],
                        start=True,
                        stop=True,
                        tile_position=(0, 0),
                    )
                    nc.tensor.matmul(
                        ps[:, MT : 2 * MT],
                        k2_sb[FOUT:128, ltB * LT : (ltB + 1) * LT],
                        q2_sb[FOUT:128, mt * MT : (mt + 1) * MT],
                        start=True,
                        stop=True,
                        tile_position=(64, 0),
                    )
                    s_sb = spool.tile([128, 2 * MT], bf16, name="s_sb")
                    if p % 2 == 0:
                        nc.vector.tensor_scalar_max(s_sb, ps, 0.0)
                    else:
                        nc.scalar.activation(s_sb, ps, AF.Relu)
                    pending.append((p, s_sb))
                # software pipeline: flush the two mm2 blocks from the
                # previous pair-batch, so the PE never waits on a relu
                while len(pending) > 2:
                    flush_mm2()

            while pending:
                flush_mm2(last=(len(pending) == 1))
            o_sb = opool.tile([FOUT, MT], f32, name="o_sb")
            if mt % 2 == 0:
                nc.scalar.copy(o_sb, po)
            else:
                nc.vector.tensor_copy(o_sb, po)
            nc.sync.dma_start(out_d[:, mt * MT : (mt + 1) * MT], o_sb)

    nc.finalize()
    return nc


def _get_nc():
    if "nc" not in _NC_CACHE:
        _NC_CACHE["nc"] = _build_nc()
    return _NC_CACHE["nc"]


def make_in_maps(x, K_w, K_b, Q_w, Q_b, V_w, V_b):
    """Host-side marshalling: per-core input dicts for the SPMD kernel."""
    import ml_dtypes

    bf = ml_dtypes.bfloat16
    x = np.asarray(x, np.float32)
    # xpad col c = x col (c-1); cols 0, L+1, L+2 are zero
    xpad = np.zeros((B, FIN, L + 3), np.float32)
    xpad[:, :, 1 : L + 1] = x

    def wT(w):  # [co, ci, t] -> per-tap [ci, co]
        a = np.transpose(np.asarray(w, np.float32), (2, 1, 0))
        return a[0], a[1], a[2]

    kt0, kt1, kt2 = wT(K_w)
    qt0, qt1, qt2 = wT(Q_w)
    vt0, vt1, vt2 = (t / SQRT_KS for t in wT(V_w))
    kqw = np.zeros((128, 4 * FOUT), np.float32)
    kqw[0:FIN, 0:FOUT] = kt0
    kqw[FIN:128, 0:FOUT] = kt1
    kqw[0:FIN, FOUT : 2 * FOUT] = qt0
    kqw[FIN:128, FOUT : 2 * FOUT] = qt1
    kqw[0:FIN, 2 * FOUT : 3 * FOUT] = kt2
    kqw[0:FIN, 3 * FOUT : 4 * FOUT] = qt2
    vw = np.zeros((128, 2 * FOUT), np.float32)
    vw[0:FIN, 0:FOUT] = vt0
    vw[FIN:128, 0:FOUT] = vt1
    vw[0:FIN, FOUT : 2 * FOUT] = vt2
    kqb = np.zeros((128, 2), np.float32)
    kqb[0:FIN, 0] = np.asarray(K_b, np.float32)
    kqb[FIN:128, 0] = np.asarray(Q_b, np.float32)
    kqb[0:FIN, 1] = np.asarray(Q_b, np.float32)
    vb = (np.asarray(V_b, np.float32) / SQRT_KS).reshape(1, FOUT)

    def shift_stack(a, lo):  # [64, BLK] window + 1-shifted copy
        return np.concatenate([a[:, lo : lo + BLK], a[:, lo + 1 : lo + BLK + 1]], 0)

    cast = lambda a: np.ascontiguousarray(a.astype(bf))
    in_maps = []
    for core in range(NCORES):
        b, h = divmod(core, 2)
        own, oth = h * HALF, (1 - h) * HALF
        xd = np.concatenate(
            [shift_stack(xpad[b], own), shift_stack(xpad[b], oth)], 1
        )
        in_maps.append(
            dict(xd=cast(xd), kqw=cast(kqw), vw=cast(vw), kqb=kqb, vb=vb)
        )
    return in_maps


def assemble(results):
    out = np.empty((B, FOUT, L), np.float32)
    for b in range(B):
        # core (b, h) returns columns in [own half | other half] order
        r0 = results[2 * b]["out"]          # h=0: [0:2048 | 2048:4096] natural
        r1 = results[2 * b + 1]["out"]      # h=1: [2048:4096 | 0:2048]
        out[b, :, 0:HALF] = r0[:, 0:HALF] + r1[:, HALF:L]
        out[b, :, HALF:L] = r0[:, HALF:L] + r1[:, 0:HALF]
    return out


def kernel(x, K_w, K_b, Q_w, Q_b, V_w, V_b):
    from concourse.bass_utils import run_bass_kernel_spmd

    nc = _get_nc()
    in_maps = make_in_maps(x, K_w, K_b, Q_w, Q_b, V_w, V_b)
    res = run_bass_kernel_spmd(nc, in_maps, core_ids=list(range(NCORES)))
    return assemble(res.results)


# revision 20
# speedup vs baseline: 1.2091x; 1.2091x over previous
"""Conv-QKV self-attention (CSA) Trainium2 Bass kernel.

Reference computation (per batch b):
    k = conv1d(x, K_w, K_b); q = conv1d(x, Q_w, Q_b); v = conv1d(x, V_w, V_b)
    scores = relu(k^T q)                # [L, L], contraction over 64 channels
    out = v @ scores / sqrt(3)          # [64, L], contraction over L
Sharding: 8 cores = 4 batches x 2 row-halves (l) of the score matrix.
Each core computes k, vT for its l-half, q for the full L, a flash-style
pass over relu(k^T q) tiles, and a PARTIAL out (contraction over its
l-half).  The host sums the two partials per batch.  1/sqrt(3) is folded
into the V weights on the host.

v3 design notes (HW-trace driven):
 - everything bf16 on the PE (1 cyc/row; f32r is 2 on HW).  rel err
   ~3e-3 vs the 2e-2 gate.
 - the HAM clock gate only reaches 8/8 (2.4 GHz + MM pipelining) after
   ~3.4us of *dense* PE activity; a stalling conv phase keeps the whole
   kernel at 4/8.  So: a short warmup burst covers the DMA preamble and
   the conv phase never blocks on the Act engine (psum bufs=4, copies
   split Act/DVE).
 - K and Q convs over the core's own l-half are fused into ONE stationary
   [128, 128] (K-cols 0:64, Q-cols 64:128): one moving-x stream computes
   both.  Taps 0+1 are K-stacked on partitions (host ships x with a
   1-shifted copy on partitions 64:128), tap 2 is a K=64 second matmul.
 - x is shipped as two 2050-col blocks [own-half | other-half] so the
   same SPMD program works for both half-cores; q2/out columns are in
   block-permuted order and the host un-permutes.
 - flash loop: row-packed mm1 pairs (two K=64 l-tiles on row groups
   h0/h64 run concurrently), depth-2 software pipeline for the mm2
   accumulation, relu alternating DVE/Act, scores pool bufs=3.
"""

import numpy as np

FIN, FOUT, KS = 64, 64, 3
B, L = 4, 4096
HALF = L // 2            # per-core l range
NCORES = 8
MT = 512                 # m tile (PSUM bank free dim, fp32)
LT = 128                 # l tile (PE partition dim)
N_MT = L // MT           # 8  (full m range per core, block-permuted)
N_LT = HALF // LT        # 16 (l tiles in this core's half)
N_G = HALF // MT         # 4  (conv groups per 2048-col block)
BLK = HALF + 2           # 2050: x block incl +-1 halo
SQRT_KS = float(np.sqrt(KS))
N_WARM = 12              # PE warmup matmuls (HAM ramp + DMA preamble cover)

_NC_CACHE = {}


def _build_nc():
    from contextlib import ExitStack

    import concourse.tile as tile
    from concourse import bacc, mybir

    f32 = mybir.dt.float32
    bf16 = mybir.dt.bfloat16
    AF = mybir.ActivationFunctionType

    nc = bacc.Bacc("TRN2", target_bir_lowering=False)

    # x as two 2050-col halo blocks [own | other], each with a 1-shifted
    # copy on partitions 64:128 (K-stacked taps 0+1).  DMA-ing raw bf16.
    xd_d = nc.dram_tensor("xd", [128, 2 * BLK], bf16, kind="ExternalInput")
    # fused conv weights [128, 256]:
    #   [:,   0: 64] = [Kt0;Kt1]   [:,  64:128] = [Qt0;Qt1]
    #   [0:64,128:192] = Kt2       [0:64,192:256] = Qt2   (rows 64:128 zero)
    kqw_d = nc.dram_tensor("kqw", [128, 4 * FOUT], bf16, kind="ExternalInput")
    # v weights (1/sqrt(3) folded): [:,0:64] = [Vt0;Vt1], [0:64,64:128] = Vt2
    vw_d = nc.dram_tensor("vw", [128, 2 * FOUT], bf16, kind="ExternalInput")
    # col 0 = [K_b; Q_b]; col 1 = [Q_b; 0] (partition-0:64-aligned copy)
    kqb_d = nc.dram_tensor("kqb", [128, 2], f32, kind="ExternalInput")
    vb_d = nc.dram_tensor("vb", [1, FOUT], f32, kind="ExternalInput")
    out_d = nc.dram_tensor("out", [FOUT, L], f32, kind="ExternalOutput")

    with tile.TileContext(nc) as tc, ExitStack() as ctx:
        consts = ctx.enter_context(tc.tile_pool(name="consts", bufs=1))
        big = ctx.enter_context(tc.tile_pool(name="big", bufs=1))

        # ---- DMA preamble: x chunk 0 first (gates the warmup burst) ---
        xd_sb = consts.tile([128, 2 * BLK], bf16)
        for c in range(4):                       # own block: 4 chunks on sync q
            sl = slice(c * MT, (c + 1) * MT + (2 if c == 3 else 0))
            nc.sync.dma_start(out=xd_sb[:, sl], in_=xd_d[:, sl])
        kqw_sb = consts.tile([128, 4 * FOUT], bf16)
        nc.gpsimd.dma_start(out=kqw_sb, in_=kqw_d[:, :])
        vw_sb = consts.tile([128, 2 * FOUT], bf16)
        nc.gpsimd.dma_start(out=vw_sb, in_=vw_d[:, :])
        kqb_sb = consts.tile([128, 2], f32)
        nc.gpsimd.dma_start(out=kqb_sb, in_=kqb_d[:, :])
        vb_sb = consts.tile([128, FOUT], f32)
        nc.gpsimd.dma_start(out=vb_sb, in_=vb_d[:, :].to_broadcast([128, FOUT]))
        for c in range(4):                       # other block: gpsimd q
            lo = BLK + c * MT
            sl = slice(lo, lo + MT + (2 if c == 3 else 0))
            nc.gpsimd.dma_start(out=xd_sb[:, sl], in_=xd_d[:, sl])

        # conv outputs, duplicated on both partition halves for the
        # row-packed score matmuls
        k2_sb = big.tile([128, HALF], bf16)
        q2_sb = big.tile([128, L], bf16)
        vt_sb = big.tile([128, N_LT, FOUT], bf16)

        # ---- stage A: warmup + conv projections ----------------------
        actx = ctx.enter_context(ExitStack())
        cpool = actx.enter_context(tc.tile_pool(name="cpsum", bufs=4, space="PSUM"))
        vpool = actx.enter_context(tc.tile_pool(name="vpsum", bufs=2, space="PSUM"))

        # All-engine warmup burst on the first landed x chunk: the clock
        # gate watches chip-wide switching activity (zero/constant data
        # does NOT count), and with every engine hot it reaches 8/8 in
        # ~5.6us instead of ~30us.  Uses real (random-normal) x data.
        wscr = consts.tile([128, MT], bf16)
        for i in range(N_WARM):
            wp = cpool.tile([128, MT], f32, name="wp", tag="cp")
            # stays inside DMA chunk 0 (cols 0:512) so it starts ASAP
            nc.tensor.matmul(
                wp[:, 0:384], xd_sb[:, 0:128], xd_sb[:, 128:512],
                start=True, stop=True,
            )
            if i % 2 == 0:
                nc.vector.tensor_scalar_max(wscr, xd_sb[:, 0:MT], 0.0)
            else:
                nc.scalar.activation(wscr, xd_sb[:, 0:MT], AF.Relu)

        # fused K+Q conv over the own block -> k2 (all) + q2 cols 0:2048
        for g in range(N_G):
            p = cpool.tile([128, MT], f32, name="pkq", tag="cp")
            nc.tensor.matmul(
                p, kqw_sb[:, 0:128], xd_sb[:, g * MT : g * MT + MT],
                start=True, stop=False,
            )
            nc.tensor.matmul(
                p, kqw_sb[0:FIN, 128:256], xd_sb[0:FIN, g * MT + 2 : g * MT + 2 + MT],
                start=False, stop=True,
            )
            gsl = slice(g * MT, (g + 1) * MT)
            nc.scalar.activation(
                k2_sb[0:FOUT, gsl], p[0:FOUT, :], AF.Identity, bias=kqb_sb[0:FOUT, 0:1]
            )
            nc.vector.tensor_scalar_add(
                q2_sb[FOUT:128, gsl], p[FOUT:128, :], kqb_sb[FOUT:128, 0:1]
            )
            nc.sync.dma_start(out=k2_sb[FOUT:128, gsl], in_=k2_sb[0:FOUT, gsl])
            nc.gpsimd.dma_start(out=q2_sb[0:FOUT, gsl], in_=q2_sb[FOUT:128, gsl])

        # vT tiles: [HALF, 64] in 128-row tiles, taps 0+1 K-stacked
        for j in range(N_LT):
            pv = vpool.tile([128, FOUT], f32, name="pv", tag="pv")
            nc.tensor.matmul(
                pv, xd_sb[:, j * LT : j * LT + LT], vw_sb[:, 0:FOUT],
                start=True, stop=False,
            )
            nc.tensor.matmul(
                pv, xd_sb[0:FIN, j * LT + 2 : j * LT + 2 + LT], vw_sb[0:FIN, FOUT:128],
                start=False, stop=True,
            )
            nc.vector.tensor_add(vt_sb[:, j, :], pv, vb_sb)

        # Q-only conv over the other block -> q2 cols 2048:4096
        for g in range(N_G):
            pq = cpool.tile([128, MT], f32, name="pq", tag="cp")
            p = pq[0:FOUT, :]
            lo = BLK + g * MT
            nc.tensor.matmul(
                p, kqw_sb[:, 64:128], xd_sb[:, lo : lo + MT], start=True, stop=False
            )
            nc.tensor.matmul(
                p, kqw_sb[0:FIN, 192:256], xd_sb[0:FIN, lo + 2 : lo + 2 + MT],
                start=False, stop=True,
            )
            gsl = slice(HALF + g * MT, HALF + (g + 1) * MT)
            if g % 2 == 0:
                nc.scalar.activation(
                    q2_sb[0:FOUT, gsl], p, AF.Identity, bias=kqb_sb[0:FOUT, 1:2]
                )
            else:
                nc.vector.tensor_scalar_add(q2_sb[0:FOUT, gsl], p, kqb_sb[0:FOUT, 1:2])
            nc.sync.dma_start(out=q2_sb[FOUT:128, gsl], in_=q2_sb[0:FOUT, gsl])

        # ---- stage B: flash loop over score tiles --------------------
        actx.close()
        spsum = ctx.enter_context(tc.tile_pool(name="spsum", bufs=3, space="PSUM"))
        spool = ctx.enter_context(tc.tile_pool(name="spool", bufs=4))
        opsum = ctx.enter_context(tc.tile_pool(name="opsum", bufs=2, space="PSUM"))
        opool = ctx.enter_context(tc.tile_pool(name="opool", bufs=2))

        NPAIR = N_LT // 2  # adjacent-l-tile pairs per m-tile
        pending = []       # [(mt, pair, s_sb), ...] awaiting their mm2 pairs
        po_tiles = {}

        def flush_mm2():
            mt_, p_, ps_sb = pending.pop(0)
            if p_ == 0:
                po_tiles[mt_] = opsum.tile([FOUT, MT], f32, name="po")
            po = po_tiles[mt_]
            nc.tensor.matmul(
                po, vt_sb[:, 2 * p_, :], ps_sb[:, 0:MT],
                start=(p_ == 0), stop=False,
            )
            nc.tensor.matmul(
                po, vt_sb[:, 2 * p_ + 1, :], ps_sb[:, MT : 2 * MT],
                start=False, stop=(p_ == NPAIR - 1),
            )
            if p_ == NPAIR - 1:  # m-tile done: drain to HBM
                o_sb = opool.tile([FOUT, MT], f32, name="o_sb")
                if mt_ % 2 == 0:
                    nc.scalar.copy(o_sb, po)
                else:
                    nc.vector.tensor_copy(o_sb, po)
                nc.sync.dma_start(out_d[:, mt_ * MT : (mt_ + 1) * MT], o_sb)
                del po_tiles[mt_]

        # batch two mm1 pairs, then two mm2 singles-blocks: fewer
        # pair<->single PE mode transitions (each costs a fill/drain).
        # The pipeline runs across m-tile boundaries - no PE drain.
        for mt in range(N_MT):
            for pp in range(0, NPAIR, 2):
                for p in (pp, pp + 1):
                    ltA, ltB = 2 * p, 2 * p + 1
                    ps = spsum.tile([128, 2 * MT], f32, name="ps")
                    # mm1 pair: row-packed (K=64 each) over adjacent l-tiles
                    nc.tensor.matmul(
                        ps[:, 0:MT],
                        k2_sb[0:FOUT, ltA * LT : (ltA + 1) * LT],
                        q2_sb[0:FOUT, mt * MT : (mt + 1) * MT],
                        start=True,
                        stop=True,
                        tile_position=(0, 0),
                    )
                    nc.tensor.matmul(
                        ps[:, MT : 2 * MT],
                        k2_sb[FOUT:128, ltB * LT : (ltB + 1) * LT],
                        q2_sb[FOUT:128, mt * MT : (mt + 1) * MT],
                        start=True,
                        stop=True,
                        tile_position=(64, 0),
                    )
                    s_sb = spool.tile([128, 2 * MT], bf16, name="s_sb")
                    if p % 2 == 0:
                        nc.vector.tensor_scalar_max(s_sb, ps, 0.0)
                    else:
                        nc.scalar.activation(s_sb, ps, AF.Relu)
                    pending.append((mt, p, s_sb))
                while len(pending) > 2:
                    flush_mm2()
        while pending:
            flush_mm2()

    nc.finalize()
    return nc


def _get_nc():
    if "nc" not in _NC_CACHE:
        _NC_CACHE["nc"] = _build_nc()
    return _NC_CACHE["nc"]


def make_in_maps(x, K_w, K_b, Q_w, Q_b, V_w, V_b):
    """Host-side marshalling: per-core input dicts for the SPMD kernel."""
    import ml_dtypes

    bf = ml_dtypes.bfloat16
    x = np.asarray(x, np.float32)
    # xpad col c = x col (c-1); cols 0, L+1, L+2 are zero
    xpad = np.zeros((B, FIN, L + 3), np.float32)
    xpad[:, :, 1 : L + 1] = x

    def wT(w):  # [co, ci, t] -> per-tap [ci, co]
        a = np.transpose(np.asarray(w, np.float32), (2, 1, 0))
        return a[0], a[1], a[2]

    kt0, kt1, kt2 = wT(K_w)
    qt0, qt1, qt2 = wT(Q_w)
    vt0, vt1, vt2 = (t / SQRT_KS for t in wT(V_w))
    kqw = np.zeros((128, 4 * FOUT), np.float32)
    kqw[0:FIN, 0:FOUT] = kt0
    kqw[FIN:128, 0:FOUT] = kt1
    kqw[0:FIN, FOUT : 2 * FOUT] = qt0
    kqw[FIN:128, FOUT : 2 * FOUT] = qt1
    kqw[0:FIN, 2 * FOUT : 3 * FOUT] = kt2
    kqw[0:FIN, 3 * FOUT : 4 * FOUT] = qt2
    vw = np.zeros((128, 2 * FOUT), np.float32)
    vw[0:FIN, 0:FOUT] = vt0
    vw[FIN:128, 0:FOUT] = vt1
    vw[0:FIN, FOUT : 2 * FOUT] = vt2
    kqb = np.zeros((128, 2), np.float32)
    kqb[0:FIN, 0] = np.asarray(K_b, np.float32)
    kqb[FIN:128, 0] = np.asarray(Q_b, np.float32)
    kqb[0:FIN, 1] = np.asarray(Q_b, np.float32)
    vb = (np.asarray(V_b, np.float32) / SQRT_KS).reshape(1, FOUT)

    def shift_stack(a, lo):  # [64, BLK] window + 1-shifted copy
        return np.concatenate([a[:, lo : lo + BLK], a[:, lo + 1 : lo + BLK + 1]], 0)

    cast = lambda a: np.ascontiguousarray(a.astype(bf))
    in_maps = []
    for core in range(NCORES):
        b, h = divmod(core, 2)
        own, oth = h * HALF, (1 - h) * HALF
        xd = np.concatenate(
            [shift_stack(xpad[b], own), shift_stack(xpad[b], oth)], 1
        )
        in_maps.append(
            dict(xd=cast(xd), kqw=cast(kqw), vw=cast(vw), kqb=kqb, vb=vb)
        )
    return in_maps


def assemble(results):
    out = np.empty((B, FOUT, L), np.float32)
    for b in range(B):
        # core (b, h) returns columns in [own half | other half] order
        r0 = results[2 * b]["out"]          # h=0: [0:2048 | 2048:4096] natural
        r1 = results[2 * b + 1]["out"]      # h=1: [2048:4096 | 0:2048]
        out[b, :, 0:HALF] = r0[:, 0:HALF] + r1[:, HALF:L]
        out[b, :, HALF:L] = r0[:, HALF:L] + r1[:, 0:HALF]
    return out


def kernel(x, K_w, K_b, Q_w, Q_b, V_w, V_b):
    from concourse.bass_utils import run_bass_kernel_spmd

    nc = _get_nc()
    in_maps = make_in_maps(x, K_w, K_b, Q_w, Q_b, V_w, V_b)
    res = run_bass_kernel_spmd(nc, in_maps, core_ids=list(range(NCORES)))
    return assemble(res.results)
